# revision 5
# baseline (speedup 1.0000x reference)
"""Trainium2 Bass kernel for BasicBlock(1w4a): quant-act conv3x3 + BN + ReLU.

Data-parallel over 8 NeuronCores (batch 32 -> 8 x 4). Each core packs 2
samples onto the 128 SBUF partitions (64 channels each) and runs the 3x3
conv as shifted matmuls with block-diagonal weights accumulating in PSUM.

Exactness: activations quantize to integers 0..15, weights binarize to +-1.
Both are exact in bf16/fp8, and PSUM accumulates in fp32 (sums bounded well
below 2^24), so the conv itself is bit-exact. The DoReFa scale (alpha/15)
and BatchNorm fold into a per-channel affine applied by ScalarE as
relu(scale*psum + bias).

fp8dr variant: fp8e4m3 quantized grid; the (dh=-1, dw) and (dh=+1, dw) taps
pair into one DoubleRow matmul (K virtualized to 256, rhs middle-dim step =
2 grid rows = 256 B), the (dh=0, dw) taps stay as normal matmuls -> 6
matmuls per 4-row output chunk instead of 9.
"""

import os

import numpy as np
import ml_dtypes

import concourse.bass as bass
import concourse.mybir as mybir
import concourse.tile as tile
from concourse import bacc
from concourse.bass_utils import run_bass_kernel_spmd

# ---- problem constants (hardcoded per harness contract) ----
N_CORES = 8
B_FULL = 32
B_SHARD = B_FULL // N_CORES  # 4
C = 64
H = 112
W = 112
BN_EPS = 1e-5

P = 128           # SBUF partitions
GW = 128          # padded grid row width (112 data + 16 zero pad)
RPB = 28          # output rows per block
GR = RPB + 2      # grid rows per block incl halo
NBLK = H // RPB   # 4 blocks per sample-pair
NCH = RPB // 4    # 4-row PSUM chunks per block
HDR = 16          # zero header elems (catches tap reads at flat index -1)
GRID = GR * GW    # 3840
TRL = 32          # zero trailer elems (catches tap reads past the grid)
STORE = HDR + GRID + TRL

MAGIC = 12582912.0  # 1.5 * 2^23: x+MAGIC-MAGIC rounds to int, half-to-even

# variant: "bf16" = 9 plain matmuls; "fp8dr" = fp8 with DoubleRow tap pairs
VARIANT = os.environ.get("KERNEL_VARIANT", "fp8dr")

_cache = {}


def _build_nc(variant):
    fp8 = variant == "fp8dr"
    qdt = mybir.dt.float8e4 if fp8 else mybir.dt.bfloat16

    nc = bacc.Bacc(None, target_bir_lowering=False)
    x = nc.dram_tensor("x", [B_SHARD, C, H, W], mybir.dt.float32,
                       kind="ExternalInput")
    scale_d = nc.dram_tensor("scale", [P, 1], mybir.dt.float32,
                             kind="ExternalInput")
    bias_d = nc.dram_tensor("bias", [P, 1], mybir.dt.float32,
                            kind="ExternalInput")
    if fp8:
        # 3 DoubleRow pair sets [(dh=-1,dw),(dh=+1,dw)] and 3 singles (dh=0)
        wdr_d = nc.dram_tensor("wdr", [P, 3 * 2 * P], mybir.dt.float8e4,
                               kind="ExternalInput")
        wsg_d = nc.dram_tensor("wsg", [P, 3 * P], mybir.dt.float8e4,
                               kind="ExternalInput")
    else:
        w9_d = nc.dram_tensor("w9", [P, 9 * P], mybir.dt.bfloat16,
                              kind="ExternalInput")
    y = nc.dram_tensor("y", [B_SHARD, C, H, W], mybir.dt.float32,
                       kind="ExternalOutput")

    with tile.TileContext(nc) as tc:
        with (
            tc.tile_pool(name="singles", bufs=1) as singles,
            tc.tile_pool(name="raws", bufs=4) as raw_pool,
            tc.tile_pool(name="qgs", bufs=4) as qg_pool,
            tc.tile_pool(name="outs", bufs=4) as out_pool,
            tc.tile_pool(name="psums", bufs=8, space="PSUM") as psum_pool,
        ):
            if fp8:
                wdr_t = singles.tile([P, 3 * 2 * P], mybir.dt.float8e4)
                nc.sync.dma_start(out=wdr_t[:, :], in_=wdr_d[:, :])
                wsg_t = singles.tile([P, 3 * P], mybir.dt.float8e4)
                nc.sync.dma_start(out=wsg_t[:, :], in_=wsg_d[:, :])
            else:
                w9_t = singles.tile([P, 9 * P], mybir.dt.bfloat16)
                nc.sync.dma_start(out=w9_t[:, :], in_=w9_d[:, :])
            scale_t = singles.tile([P, 1], mybir.dt.float32)
            nc.sync.dma_start(out=scale_t[:, :], in_=scale_d[:, :])
            bias_t = singles.tile([P, 1], mybir.dt.float32)
            nc.sync.dma_start(out=bias_t[:, :], in_=bias_d[:, :])

            for pair in range(B_SHARD // 2):
                x2 = x[2 * pair:2 * pair + 2].rearrange(
                    "s c h w -> (s c) h w")      # [128, H, W]
                y2 = y[2 * pair:2 * pair + 2].rearrange(
                    "s c h w -> (s c) h w")
                for blk in range(NBLK):
                    r0 = RPB * blk
                    lo = max(r0 - 1, 0)
                    hi = min(r0 + RPB, H - 1)
                    cnt = hi - lo + 1
                    dst_off = lo - (r0 - 1)

                    raw = raw_pool.tile([P, GR, W], mybir.dt.float32)
                    qg = qg_pool.tile([P, STORE], qdt)
                    nc.gpsimd.memset(qg[:, 0:HDR], 0.0)
                    nc.gpsimd.memset(qg[:, HDR + GRID:STORE], 0.0)
                    qg3 = qg[:, HDR:HDR + GRID].rearrange(
                        "p (r c) -> p r c", c=GW)
                    nc.gpsimd.memset(qg3[:, :, W:GW], 0.0)

                    # quantize in two half-blocks for finer pipelining
                    SR = GR // 2
                    for (a, b) in ((0, SR), (SR, GR)):
                        da = max(a, dst_off)
                        db = min(b, dst_off + cnt)
                        nc.sync.dma_start(
                            out=raw[:, da:db, :],
                            in_=x2[:, lo + (da - dst_off):lo + (db - dst_off), :],
                        )
                        if blk == 0 and a == 0:
                            nc.vector.memset(raw[:, 0, :], 0.0)
                        if blk == NBLK - 1 and b == GR:
                            nc.vector.memset(raw[:, GR - 1, :], 0.0)
                        rawh = raw[:, a:b, :].rearrange("p a b -> p (a b)")
                        # t = max(15*x, 0)
                        nc.vector.tensor_scalar(
                            out=rawh, in0=rawh,
                            scalar1=15.0, scalar2=0.0,
                            op0=mybir.AluOpType.mult, op1=mybir.AluOpType.max,
                        )
                        # t = min(t,15) + MAGIC (fp32 add rounds to int, RNE)
                        nc.gpsimd.tensor_scalar(
                            out=rawh, in0=rawh,
                            scalar1=15.0, scalar2=MAGIC,
                            op0=mybir.AluOpType.min, op1=mybir.AluOpType.add,
                        )
                        # q = t - MAGIC -> integer 0..15, exact in bf16/fp8
                        nc.vector.tensor_scalar(
                            out=qg3[:, a:b, 0:W], in0=raw[:, a:b, :],
                            scalar1=MAGIC, scalar2=None,
                            op0=mybir.AluOpType.subtract,
                        )

                    ot = out_pool.tile([P, RPB, W], mybir.dt.float32)
                    for ch in range(NCH):
                        ps = psum_pool.tile([P, 512], mybir.dt.float32,
                                            name=f"ps{pair}_{blk}_{ch}",
                                            tag="ps")
                        if fp8:
                            # 3 DoubleRow pair-matmuls: taps (-1,dw)+(+1,dw)
                            for i, dw in enumerate((-1, 0, 1)):
                                base = HDR + (4 * ch) * GW + dw
                                rhs = qg[:, base:base + 512]
                                v = rhs.ap
                                v[1] = [2 * GW, 2]
                                v.append([1, 512])
                                rhs.ap = v
                                lhsT = wdr_t[:, i * 2 * P:(i + 1) * 2 * P] \
                                    .rearrange("p (a b) -> p a b", a=2)
                                nc.tensor.matmul(
                                    ps[:, :], lhsT=lhsT, rhs=rhs,
                                    start=(i == 0), stop=False,
                                    perf_mode=mybir.MatmulPerfMode.DoubleRow,
                                )
                            # 3 normal matmuls: taps (0, dw)
                            for i, dw in enumerate((-1, 0, 1)):
                                base = HDR + (4 * ch + 1) * GW + dw
                                nc.tensor.matmul(
                                    ps[:, :],
                                    lhsT=wsg_t[:, i * P:(i + 1) * P],
                                    rhs=qg[:, base:base + 512],
                                    start=False, stop=(i == 2),
                                )
                        else:
                            for t in range(9):
                                dh, dw = t // 3 - 1, t % 3 - 1
                                base = HDR + (4 * ch + 1 + dh) * GW + dw
                                nc.tensor.matmul(
                                    ps[:, :],
                                    lhsT=w9_t[:, t * P:(t + 1) * P],
                                    rhs=qg[:, base:base + 512],
                                    start=(t == 0), stop=(t == 8),
                                )
                        pv = ps.rearrange("p (r c) -> p r c", c=GW)
                        nc.scalar.activation(
                            out=ot[:, 4 * ch:4 * ch + 4, :],
                            in_=pv[:, :, 0:W],
                            func=mybir.ActivationFunctionType.Relu,
                            bias=bias_t[:, 0:1],
                            scale=scale_t[:, 0:1],
                        )
                    nc.sync.dma_start(
                        out=y2[:, r0:r0 + RPB, :],
                        in_=ot[:, :, :],
                    )

    nc.finalize()
    return nc


def _blockdiag(blk64):
    out = np.zeros((P, P), dtype=np.float32)
    out[0:64, 0:64] = blk64
    out[64:128, 64:128] = blk64
    return out


def _host_prep(w, gamma, beta, bn_mean, bn_var, variant):
    w = np.asarray(w, dtype=np.float32)
    alpha = np.float32(np.mean(np.abs(w)))
    ws = np.sign(w).astype(np.float32)           # [co, ci, 3, 3]
    inv = (np.asarray(gamma, np.float32)
           / np.sqrt(np.asarray(bn_var, np.float32) + np.float32(BN_EPS)))
    scale_c = (inv * (alpha / np.float32(15.0))).astype(np.float32)
    bias_c = (np.asarray(beta, np.float32)
              - np.asarray(bn_mean, np.float32) * inv).astype(np.float32)
    scale128 = np.ascontiguousarray(
        np.concatenate([scale_c, scale_c]).reshape(P, 1))
    bias128 = np.ascontiguousarray(
        np.concatenate([bias_c, bias_c]).reshape(P, 1))

    wm = {}
    if variant == "fp8dr":
        wdr = np.zeros((P, 3, 2, P), dtype=np.float32)
        wsg = np.zeros((P, 3, P), dtype=np.float32)
        for i, dw in enumerate(range(3)):
            wdr[:, i, 0, :] = _blockdiag(ws[:, :, 0, dw].T)  # dh=-1
            wdr[:, i, 1, :] = _blockdiag(ws[:, :, 2, dw].T)  # dh=+1
            wsg[:, i, :] = _blockdiag(ws[:, :, 1, dw].T)     # dh=0
        wm["wdr"] = np.ascontiguousarray(
            wdr.reshape(P, 6 * P).astype(ml_dtypes.float8_e4m3))
        wm["wsg"] = np.ascontiguousarray(
            wsg.reshape(P, 3 * P).astype(ml_dtypes.float8_e4m3))
    else:
        w9 = np.zeros((P, 9, P), dtype=np.float32)
        for t in range(9):
            dh, dw = t // 3, t % 3
            w9[:, t, :] = _blockdiag(ws[:, :, dh, dw].T)
        wm["w9"] = np.ascontiguousarray(
            w9.reshape(P, 9 * P).astype(ml_dtypes.bfloat16))
    return wm, scale128, bias128


_last_results = None  # test harness peeks at this for profile data


def kernel(x, w, gamma, beta, bn_mean, bn_var):
    global _last_results
    variant = VARIANT
    if variant not in _cache:
        _cache[variant] = _build_nc(variant)
    nc = _cache[variant]

    wm, scale128, bias128 = _host_prep(w, gamma, beta, bn_mean, bn_var,
                                       variant)
    x = np.asarray(x, dtype=np.float32)

    in_maps = []
    for i in range(N_CORES):
        m = {
            "x": np.ascontiguousarray(x[i * B_SHARD:(i + 1) * B_SHARD]),
            "scale": scale128,
            "bias": bias128,
        }
        m.update(wm)
        in_maps.append(m)
    res = run_bass_kernel_spmd(nc, in_maps, core_ids=list(range(N_CORES)))
    _last_results = res
    return np.concatenate([res.results[i]["y"] for i in range(N_CORES)],
                          axis=0)


# revision 6
# speedup vs baseline: 3.8960x; 3.8960x over previous
"""Trainium2 Bass kernel for BasicBlock(1w4a): quant-act conv3x3 + BN + ReLU.

Data-parallel over 8 NeuronCores (batch 32 -> 8 x 4). Each core packs 2
samples onto the 128 SBUF partitions (64 channels each) and runs the 3x3
conv as shifted matmuls with block-diagonal weights accumulating in PSUM.

Exactness: activations quantize to integers 0..15, weights binarize to +-1.
Both are exact in bf16/fp8, and PSUM accumulates in fp32 (sums bounded well
below 2^24), so the conv itself is bit-exact. The DoReFa scale (alpha/15)
and BatchNorm fold into a per-channel affine applied by ScalarE as
relu(scale*psum + bias).

fp8dr variant: fp8e4m3 quantized grid; the (dh=-1, dw) and (dh=+1, dw) taps
pair into one DoubleRow matmul (K virtualized to 256, rhs middle-dim step =
2 grid rows = 256 B), the (dh=0, dw) taps stay as normal matmuls -> 6
matmuls per 4-row output chunk instead of 9.
"""

import os

import numpy as np
import ml_dtypes

import concourse.bass as bass
import concourse.mybir as mybir
import concourse.tile as tile
from concourse import bacc
from concourse.bass_utils import run_bass_kernel_spmd

# ---- problem constants (hardcoded per harness contract) ----
N_CORES = 8
B_FULL = 32
B_SHARD = B_FULL // N_CORES  # 4
C = 64
H = 112
W = 112
BN_EPS = 1e-5

P = 128           # SBUF partitions
GW = 128          # padded grid row width (112 data + 16 zero pad)
RPB = 28          # output rows per block
GR = RPB + 2      # grid rows per block incl halo
NBLK = H // RPB   # 4 blocks per sample-pair
NCH = RPB // 4    # 4-row PSUM chunks per block
HDR = 16          # zero header elems (catches tap reads at flat index -1)
GRID = GR * GW    # 3840
TRL = 32          # zero trailer elems (catches tap reads past the grid)
STORE = HDR + GRID + TRL

MAGIC = 12582912.0  # 1.5 * 2^23: x+MAGIC-MAGIC rounds to int, half-to-even

# variant: "bf16" = 9 plain matmuls; "fp8dr" = fp8 with DoubleRow tap pairs
VARIANT = os.environ.get("KERNEL_VARIANT", "fp8dr")

_cache = {}


def _build_nc(variant):
    fp8 = variant == "fp8dr"
    qdt = mybir.dt.float8e4 if fp8 else mybir.dt.bfloat16

    nc = bacc.Bacc(None, target_bir_lowering=False)
    x = nc.dram_tensor("x", [B_SHARD, C, H, W], mybir.dt.float32,
                       kind="ExternalInput")
    scale_d = nc.dram_tensor("scale", [P, 1], mybir.dt.float32,
                             kind="ExternalInput")
    bias_d = nc.dram_tensor("bias", [P, 1], mybir.dt.float32,
                            kind="ExternalInput")
    if fp8:
        # 3 DoubleRow pair sets [(dh=-1,dw),(dh=+1,dw)] and 3 singles (dh=0)
        wdr_d = nc.dram_tensor("wdr", [P, 3 * 2 * P], mybir.dt.float8e4,
                               kind="ExternalInput")
        wsg_d = nc.dram_tensor("wsg", [P, 3 * P], mybir.dt.float8e4,
                               kind="ExternalInput")
    else:
        w9_d = nc.dram_tensor("w9", [P, 9 * P], mybir.dt.bfloat16,
                              kind="ExternalInput")
    y = nc.dram_tensor("y", [B_SHARD, C, H, W], mybir.dt.float32,
                       kind="ExternalOutput")

    with tile.TileContext(nc) as tc:
        with (
            tc.tile_pool(name="singles", bufs=1) as singles,
            tc.tile_pool(name="raws", bufs=4) as raw_pool,
            tc.tile_pool(name="qgs", bufs=4) as qg_pool,
            tc.tile_pool(name="outs", bufs=4) as out_pool,
            tc.tile_pool(name="psums", bufs=8, space="PSUM") as psum_pool,
        ):
            if fp8:
                wdr_t = singles.tile([P, 3 * 2 * P], mybir.dt.float8e4)
                nc.sync.dma_start(out=wdr_t[:, :], in_=wdr_d[:, :])
                wsg_t = singles.tile([P, 3 * P], mybir.dt.float8e4)
                nc.sync.dma_start(out=wsg_t[:, :], in_=wsg_d[:, :])
            else:
                w9_t = singles.tile([P, 9 * P], mybir.dt.bfloat16)
                nc.sync.dma_start(out=w9_t[:, :], in_=w9_d[:, :])
            scale_t = singles.tile([P, 1], mybir.dt.float32)
            nc.sync.dma_start(out=scale_t[:, :], in_=scale_d[:, :])
            bias_t = singles.tile([P, 1], mybir.dt.float32)
            nc.sync.dma_start(out=bias_t[:, :], in_=bias_d[:, :])

            for pair in range(B_SHARD // 2):
                x2 = x[2 * pair:2 * pair + 2].rearrange(
                    "s c h w -> (s c) h w")      # [128, H, W]
                y2 = y[2 * pair:2 * pair + 2].rearrange(
                    "s c h w -> (s c) h w")
                for blk in range(NBLK):
                    r0 = RPB * blk
                    lo = max(r0 - 1, 0)
                    hi = min(r0 + RPB, H - 1)
                    cnt = hi - lo + 1
                    dst_off = lo - (r0 - 1)

                    raw = raw_pool.tile([P, GR, W], mybir.dt.float32)
                    qg = qg_pool.tile([P, STORE], qdt)
                    nc.gpsimd.memset(qg[:, 0:HDR], 0.0)
                    nc.gpsimd.memset(qg[:, HDR + GRID:STORE], 0.0)
                    qg3 = qg[:, HDR:HDR + GRID].rearrange(
                        "p (r c) -> p r c", c=GW)
                    nc.gpsimd.memset(qg3[:, :, W:GW], 0.0)

                    # quantize in two half-blocks for finer pipelining
                    SR = GR // 2
                    for (a, b) in ((0, SR), (SR, GR)):
                        da = max(a, dst_off)
                        db = min(b, dst_off + cnt)
                        nc.sync.dma_start(
                            out=raw[:, da:db, :],
                            in_=x2[:, lo + (da - dst_off):lo + (db - dst_off), :],
                        )
                        if blk == 0 and a == 0:
                            nc.vector.memset(raw[:, 0, :], 0.0)
                        if blk == NBLK - 1 and b == GR:
                            nc.vector.memset(raw[:, GR - 1, :], 0.0)
                        rawh = raw[:, a:b, :].rearrange("p a b -> p (a b)")
                        # t = max(15*x, 0)
                        nc.vector.tensor_scalar(
                            out=rawh, in0=rawh,
                            scalar1=15.0, scalar2=0.0,
                            op0=mybir.AluOpType.mult, op1=mybir.AluOpType.max,
                        )
                        # t = min(t,15) + MAGIC (fp32 add rounds to int, RNE)
                        nc.vector.tensor_scalar(
                            out=rawh, in0=rawh,
                            scalar1=15.0, scalar2=MAGIC,
                            op0=mybir.AluOpType.min, op1=mybir.AluOpType.add,
                        )
                        # q = t - MAGIC -> integer 0..15, exact in bf16/fp8
                        nc.vector.tensor_scalar(
                            out=qg3[:, a:b, 0:W], in0=raw[:, a:b, :],
                            scalar1=MAGIC, scalar2=None,
                            op0=mybir.AluOpType.subtract,
                        )

                    ot = out_pool.tile([P, RPB, W], mybir.dt.float32)
                    for ch in range(NCH):
                        ps = psum_pool.tile([P, 512], mybir.dt.float32,
                                            name=f"ps{pair}_{blk}_{ch}",
                                            tag="ps")
                        if fp8:
                            # 3 DoubleRow pair-matmuls: taps (-1,dw)+(+1,dw)
                            for i, dw in enumerate((-1, 0, 1)):
                                base = HDR + (4 * ch) * GW + dw
                                rhs = qg[:, base:base + 512]
                                v = rhs.ap
                                v[1] = [2 * GW, 2]
                                v.append([1, 512])
                                rhs.ap = v
                                lhsT = wdr_t[:, i * 2 * P:(i + 1) * 2 * P] \
                                    .rearrange("p (a b) -> p a b", a=2)
                                nc.tensor.matmul(
                                    ps[:, :], lhsT=lhsT, rhs=rhs,
                                    start=(i == 0), stop=False,
                                    perf_mode=mybir.MatmulPerfMode.DoubleRow,
                                )
                            # 3 normal matmuls: taps (0, dw)
                            for i, dw in enumerate((-1, 0, 1)):
                                base = HDR + (4 * ch + 1) * GW + dw
                                nc.tensor.matmul(
                                    ps[:, :],
                                    lhsT=wsg_t[:, i * P:(i + 1) * P],
                                    rhs=qg[:, base:base + 512],
                                    start=False, stop=(i == 2),
                                )
                        else:
                            for t in range(9):
                                dh, dw = t // 3 - 1, t % 3 - 1
                                base = HDR + (4 * ch + 1 + dh) * GW + dw
                                nc.tensor.matmul(
                                    ps[:, :],
                                    lhsT=w9_t[:, t * P:(t + 1) * P],
                                    rhs=qg[:, base:base + 512],
                                    start=(t == 0), stop=(t == 8),
                                )
                        pv = ps.rearrange("p (r c) -> p r c", c=GW)
                        nc.scalar.activation(
                            out=ot[:, 4 * ch:4 * ch + 4, :],
                            in_=pv[:, :, 0:W],
                            func=mybir.ActivationFunctionType.Relu,
                            bias=bias_t[:, 0:1],
                            scale=scale_t[:, 0:1],
                        )
                    nc.sync.dma_start(
                        out=y2[:, r0:r0 + RPB, :],
                        in_=ot[:, :, :],
                    )

    nc.finalize()
    return nc


def _blockdiag(blk64):
    out = np.zeros((P, P), dtype=np.float32)
    out[0:64, 0:64] = blk64
    out[64:128, 64:128] = blk64
    return out


def _host_prep(w, gamma, beta, bn_mean, bn_var, variant):
    w = np.asarray(w, dtype=np.float32)
    alpha = np.float32(np.mean(np.abs(w)))
    ws = np.sign(w).astype(np.float32)           # [co, ci, 3, 3]
    inv = (np.asarray(gamma, np.float32)
           / np.sqrt(np.asarray(bn_var, np.float32) + np.float32(BN_EPS)))
    scale_c = (inv * (alpha / np.float32(15.0))).astype(np.float32)
    bias_c = (np.asarray(beta, np.float32)
              - np.asarray(bn_mean, np.float32) * inv).astype(np.float32)
    scale128 = np.ascontiguousarray(
        np.concatenate([scale_c, scale_c]).reshape(P, 1))
    bias128 = np.ascontiguousarray(
        np.concatenate([bias_c, bias_c]).reshape(P, 1))

    wm = {}
    if variant == "fp8dr":
        wdr = np.zeros((P, 3, 2, P), dtype=np.float32)
        wsg = np.zeros((P, 3, P), dtype=np.float32)
        for i, dw in enumerate(range(3)):
            wdr[:, i, 0, :] = _blockdiag(ws[:, :, 0, dw].T)  # dh=-1
            wdr[:, i, 1, :] = _blockdiag(ws[:, :, 2, dw].T)  # dh=+1
            wsg[:, i, :] = _blockdiag(ws[:, :, 1, dw].T)     # dh=0
        wm["wdr"] = np.ascontiguousarray(
            wdr.reshape(P, 6 * P).astype(ml_dtypes.float8_e4m3))
        wm["wsg"] = np.ascontiguousarray(
            wsg.reshape(P, 3 * P).astype(ml_dtypes.float8_e4m3))
    else:
        w9 = np.zeros((P, 9, P), dtype=np.float32)
        for t in range(9):
            dh, dw = t // 3, t % 3
            w9[:, t, :] = _blockdiag(ws[:, :, dh, dw].T)
        wm["w9"] = np.ascontiguousarray(
            w9.reshape(P, 9 * P).astype(ml_dtypes.bfloat16))
    return wm, scale128, bias128


_last_results = None  # test harness peeks at this for profile data


def kernel(x, w, gamma, beta, bn_mean, bn_var):
    global _last_results
    variant = VARIANT
    if variant not in _cache:
        _cache[variant] = _build_nc(variant)
    nc = _cache[variant]

    wm, scale128, bias128 = _host_prep(w, gamma, beta, bn_mean, bn_var,
                                       variant)
    x = np.asarray(x, dtype=np.float32)

    in_maps = []
    for i in range(N_CORES):
        m = {
            "x": np.ascontiguousarray(x[i * B_SHARD:(i + 1) * B_SHARD]),
            "scale": scale128,
            "bias": bias128,
        }
        m.update(wm)
        in_maps.append(m)
    res = run_bass_kernel_spmd(nc, in_maps, core_ids=list(range(N_CORES)))
    _last_results = res
    return np.concatenate([res.results[i]["y"] for i in range(N_CORES)],
                          axis=0)
